# revision 24
# baseline (speedup 1.0000x reference)
"""Trainium2 Bass kernel for the DepGraph relaxed-Bernoulli sampler.

Computes, for full inputs:
  A  = sigmoid((logitexp(-0.5*d2(uM,uR)/exp(g)) + logit(noise_A)) / T)   [16384, 4096]
  G  = unsort(triu_strict_mask * sigmoid((logitexp(-0.5*d2(Y,Y)/exp(g)) + logit(noise_G)) / T))
with Y = uR sorted by sum(log Phi(uR)) and the output unsorted on both axes.

Device math (validated against the jax reference to ~2e-6 absmax):
  na  = k*|x|^2 + k*|y|^2 - 2k*(x.y)        (k = 0.5*exp(-g); via K=66 augmented matmul)
  lmc = ln(max(exp(na) - 1, 1e-20))         ( = -logitexp(-na) )
  w   = ln(u) - ln(1-u) - lmc
  out = sigmoid(w / T)

Sharding: A rows in contiguous 2048-row slabs per core; G computed in sorted
space with rows c::8 per core (balances the upper-triangular work), lower
block-triangle skipped (outputs are pre-zeroed by the runner), host applies the
inverse permutation on both axes afterwards.
"""

import math
from contextlib import ExitStack

import numpy as np

import concourse.bass as bass
import concourse.tile as tile
from concourse import mybir
from concourse.bass_utils import run_bass_kernel_spmd

M, N, D = 16384, 4096, 64
NCORES = 8
MLOC = M // NCORES          # 2048 A rows per core
GLOC = N // NCORES          # 512 G rows per core
KAUG = D + 2                # 66
P = 128                     # partitions
CT = 512                    # psum col tile (one bank of f32)
HW = 2048                   # elementwise half-block width
NBLK_A = MLOC // P          # 16
NBLK_G = GLOC // P          # 4
TEMP = 0.3
CHUNK = 4                   # blocks per ACT-table phase chunk

F32 = mybir.dt.float32
ACT = mybir.ActivationFunctionType
ALU = mybir.AluOpType

# column layout of the packed per-core matmul-constants tensor [KAUG, CW]
C_LHSA = 0                     # [:, :MLOC]   augmented uM slab, transposed
C_RHSA = C_LHSA + MLOC         # [:, :N]      augmented uR, transposed
C_LHSG = C_RHSA + N            # [:, :GLOC]   augmented sorted-Y slab (rows c::8)
C_RHSG = C_LHSG + GLOC         # [:, :N]      augmented sorted-Y, transposed
CW = C_RHSG + N

_PROGRAM = None


def _insts(x):
    return getattr(x, "ins", x)


def _patch_act_tables():
    # The act-table-load inserter greedily maps Exp -> exp_and_others and
    # Ln -> natural_log, paying a ~2.7us table switch on every Exp<->Ln
    # transition.  Strip exp/ln from every set except the combined
    # natural_log_exp_and_others (dict order preserved, so set ids still
    # match act_info.json) so both resolve to the one set.
    import concourse.bacc as bacc_mod

    orig = bacc_mod.get_activation_tables
    if getattr(bacc_mod.get_activation_tables, "_lnexp_patched", False):
        return
    both = {ACT.Exp, ACT.Ln}

    def patched(arch):
        t = orig(arch)
        return {
            name: (funcs if name == "natural_log_exp_and_others" else funcs - both)
            for name, funcs in t.items()
        }

    patched._lnexp_patched = True
    bacc_mod.get_activation_tables = patched


def _build_program():
    from concourse.bacc import Bacc

    _patch_act_tables()
    nc = Bacc()
    constsd = nc.declare_dram_parameter("consts", [KAUG, CW], F32, isOutput=False)
    maskd = nc.declare_dram_parameter("maskG", [P, 2 * CT], F32, isOutput=False)
    noiseA = nc.declare_dram_parameter("noiseA", [MLOC, N], F32, isOutput=False)
    noiseG = nc.declare_dram_parameter("noiseG", [GLOC, N], F32, isOutput=False)
    outA = nc.declare_dram_parameter("outA", [MLOC, N], F32, isOutput=True)
    outG = nc.declare_dram_parameter("outG", [GLOC, N], F32, isOutput=True)

    from concourse.tile_rust import add_dep_helper

    with tile.TileContext(nc) as tc, ExitStack() as ctx:
        consts = ctx.enter_context(tc.tile_pool(name="consts", bufs=1))
        consts_sb = consts.tile([KAUG, CW], F32)
        maskG_sb = consts.tile([P, 2 * CT], F32)
        # A-matmul constants land first so the first matmuls aren't gated on
        # the (colder) G constants and mask; the G part is DMA'd lazily right
        # before the first G block is traced.
        nc.sync.dma_start(out=consts_sb[:, :C_LHSG], in_=constsd[:, :C_LHSG])
        g_consts_loaded = [False]

        def _load_g_consts():
            if not g_consts_loaded[0]:
                g_consts_loaded[0] = True
                nc.sync.dma_start(
                    out=consts_sb[:, C_LHSG:], in_=constsd[:, C_LHSG:]
                )
                nc.sync.dma_start(out=maskG_sb, in_=maskd[:, :])

        lhsA_sb = consts_sb[:, C_LHSA : C_LHSA + MLOC]
        rhsA_sb = consts_sb[:, C_RHSA : C_RHSA + N]
        lhsG_sb = consts_sb[:, C_LHSG : C_LHSG + GLOC]
        rhsG_sb = consts_sb[:, C_RHSG : C_RHSG + N]

        psum = ctx.enter_context(tc.tile_pool(name="psum", bufs=2, space="PSUM"))
        upool = ctx.enter_context(tc.tile_pool(name="u", bufs=3))
        mpool = ctx.enter_context(tc.tile_pool(name="m", bufs=3))
        wpool = ctx.enter_context(tc.tile_pool(name="w", bufs=CHUNK + 1))

        blocks = [("A", b) for b in range(NBLK_A)] + [("G", t) for t in range(NBLK_G)]
        prev_sig_insts = []
        for cstart in range(0, len(blocks), CHUNK):
            chunk = blocks[cstart : cstart + CHUNK]
            p1_act_insts = []
            todo = []
            for kind, b in chunk:
                if kind == "A":
                    col0 = 0
                    width = N
                    noise_dram = noiseA[b * P : (b + 1) * P, :]
                    lhs_sb = lhsA_sb[:, b * P : (b + 1) * P]
                    rhs_sb = rhsA_sb
                    out_dram = outA[b * P : (b + 1) * P, :]
                else:
                    _load_g_consts()
                    col0 = 2 * CT * b
                    width = N - col0
                    noise_dram = noiseG[b * P : (b + 1) * P, col0:]
                    lhs_sb = lhsG_sb[:, b * P : (b + 1) * P]
                    rhs_sb = rhsG_sb
                    out_dram = outG[b * P : (b + 1) * P, col0:]

                w = wpool.tile([P, width], F32, tag="w")
                h0 = 0
                while h0 < width:
                    hw = min(HW, width - h0)
                    wsl = w[:, h0 : h0 + hw]

                    u = upool.tile([P, hw], F32, tag="u")
                    nc.sync.dma_start(out=u, in_=noise_dram[:, h0 : h0 + hw])
                    # ln(u) goes straight into the w slice; the final combine
                    # subtracts ln((1-u)*mc) from it in place.
                    i1 = nc.scalar.activation(out=wsl, in_=u, func=ACT.Ln)

                    m = mpool.tile([P, hw], F32, tag="m")
                    exp_insts = []
                    ps = psum.tile([P, hw], F32, tag="ps")
                    for j in range(hw // CT):
                        gcol = col0 + h0 + j * CT
                        nc.tensor.matmul(
                            ps[:, j * CT : (j + 1) * CT],
                            lhs_sb,
                            rhs_sb[:, gcol : gcol + CT],
                            start=True,
                            stop=True,
                        )
                    ie = nc.scalar.activation(out=m, in_=ps, func=ACT.Exp)
                    exp_insts.append(ie)
                    # mc = max(exp(na)-1, eps); q = (u-1)*mc; lv = ln(-q) =
                    # ln((1-u)*mc) = ln(1-u) + ln(mc) -- one ACT ln instead of
                    # two, and the stt fuses the (u-1) with the multiply.
                    nc.vector.tensor_scalar(
                        out=m,
                        in0=m,
                        scalar1=1.0,
                        scalar2=1e-20,
                        op0=ALU.subtract,
                        op1=ALU.max,
                    )
                    nc.vector.scalar_tensor_tensor(
                        out=m,
                        in0=u,
                        scalar=1.0,
                        in1=m,
                        op0=ALU.subtract,
                        op1=ALU.mult,
                    )
                    i3 = nc.scalar.activation(out=m, in_=m, func=ACT.Ln, scale=-1.0)
                    nc.vector.tensor_tensor(out=wsl, in0=wsl, in1=m, op=ALU.subtract)

                    p1_act_insts += [i1, i3] + exp_insts
                    h0 += hw

                todo.append((kind, w, out_dram))

            # keep the ACT instruction stream grouped by table set:
            # every phase-1 (ln/exp) op of this chunk runs after every sigmoid
            # of the previous chunk, and every sigmoid of this chunk runs after
            # every phase-1 op of this chunk.
            for pi in p1_act_insts:
                for si in prev_sig_insts:
                    add_dep_helper(_insts(pi), _insts(si), True, "act-table phase")

            sig_insts = []
            for kind, w, out_dram in todo:
                if kind == "G":
                    # maskpen is +1e30 on kept entries, -1e30 on the masked
                    # lower triangle: min() then sigmoid gives exact-ish 0
                    # there, and keeps the ACT sigmoid as the final writer
                    # (out-DMA then needs a single sem wait).
                    nc.vector.tensor_tensor(
                        out=w[:, : 2 * CT],
                        in0=w[:, : 2 * CT],
                        in1=maskG_sb,
                        op=ALU.min,
                    )
                isig = nc.scalar.activation(
                    out=w, in_=w, func=ACT.Sigmoid, scale=float(1.0 / TEMP)
                )
                for pi in p1_act_insts:
                    add_dep_helper(_insts(isig), _insts(pi), True, "act-table phase")
                sig_insts.append(isig)
                nc.sync.dma_start(out=out_dram, in_=w)
            prev_sig_insts = sig_insts

    nc.finalize()
    return nc


def _get_program():
    global _PROGRAM
    if _PROGRAM is None:
        _PROGRAM = _build_program()
    return _PROGRAM


def _ordering_sort(uR):
    # Must match the reference bitwise: jax f32 on CPU, stable argsort.
    import jax
    import jax.numpy as jnp

    cpu = jax.devices("cpu")[0]
    with jax.default_device(cpu):
        ordering = jnp.sum(
            jnp.log(0.5 + 0.5 * jax.lax.erf(jnp.asarray(uR) / np.float32(math.sqrt(2.0)))),
            axis=1,
        )
        o = np.asarray(ordering)
    sort_idx = np.argsort(o, kind="stable")
    inv = np.argsort(sort_idx, kind="stable")
    return sort_idx, inv


def _augment(X, k):
    # lhs rows: [-2k*x, k*|x|^2, 1];  rhs rows: [y, 1, k*|y|^2]
    r = np.sum(X.astype(np.float32) ** 2, axis=1, dtype=np.float32)
    ones = np.ones((X.shape[0], 1), np.float32)
    lhs = np.concatenate([(np.float32(-2.0) * k) * X, (k * r)[:, None], ones], axis=1)
    rhs = np.concatenate([X, ones, (k * r)[:, None]], axis=1)
    return (
        np.ascontiguousarray(lhs.T.astype(np.float32)),
        np.ascontiguousarray(rhs.T.astype(np.float32)),
    )


def _pack_consts(lhsA_t, rhsA, lhsG_t, rhsG, c):
    out = np.empty((KAUG, CW), np.float32)
    out[:, C_LHSA : C_LHSA + MLOC] = lhsA_t[:, c * MLOC : (c + 1) * MLOC]
    out[:, C_RHSA : C_RHSA + N] = rhsA
    out[:, C_LHSG : C_LHSG + GLOC] = lhsG_t[:, c::NCORES]
    out[:, C_RHSG : C_RHSG + N] = rhsG
    return out


def _make_mask(c):
    # +1e30 on kept entries (strict upper triangle in sorted space for this
    # core's strided rows), -1e30 where masked; min() + sigmoid zeroes those.
    jj = np.arange(2 * CT, dtype=np.float32)[None, :]
    pp = np.arange(P, dtype=np.float32)[:, None]
    return np.where(jj > (c + 8.0 * pp), np.float32(1e30), np.float32(-1e30))


def make_in_maps(uM, uR, g_logscale, noise_A, noise_G):
    uM = np.asarray(uM, np.float32)
    uR = np.asarray(uR, np.float32)
    noise_A = np.asarray(noise_A, np.float32)
    noise_G = np.asarray(noise_G, np.float32)
    g = np.float32(np.asarray(g_logscale))

    sort_idx, inv = _ordering_sort(uR)
    k = np.float32(0.5) * np.exp(-g, dtype=np.float32)

    lhsA_t, _ = _augment(uM, k)             # [66, 16384]
    _, rhsA = _augment(uR, k)               # [66, 4096]
    Y = np.ascontiguousarray(uR[sort_idx])
    lhsG_t, rhsG = _augment(Y, k)           # [66, 4096], [66, 4096]

    in_maps = []
    for c in range(NCORES):
        in_maps.append(
            {
                "consts": _pack_consts(lhsA_t, rhsA, lhsG_t, rhsG, c),
                "maskG": np.ascontiguousarray(_make_mask(c)),
                "noiseA": np.ascontiguousarray(noise_A[c * MLOC : (c + 1) * MLOC]),
                "noiseG": np.ascontiguousarray(noise_G[c::NCORES]),
            }
        )
    return in_maps, inv


def assemble_outputs(results, inv):
    A = np.concatenate([results[c]["outA"] for c in range(NCORES)], axis=0)
    Gs = np.empty((N, N), np.float32)
    for c in range(NCORES):
        Gs[c::NCORES] = results[c]["outG"]
    G = np.ascontiguousarray(Gs[inv][:, inv])
    return A, G


def kernel(uM, uR, g_logscale, noise_A, noise_G, _trace=False):
    in_maps, inv = make_in_maps(uM, uR, g_logscale, noise_A, noise_G)
    nc = _get_program()
    res = run_bass_kernel_spmd(nc, in_maps, list(range(NCORES)), trace=_trace)
    A, G = assemble_outputs(res.results, inv)
    if _trace:
        return (A, G), res
    return A, G


# revision 30
# speedup vs baseline: 187.0681x; 187.0681x over previous
"""Trainium2 Bass kernel for the DepGraph relaxed-Bernoulli sampler.

Computes, for full inputs:
  A  = sigmoid((logitexp(-0.5*d2(uM,uR)/exp(g)) + logit(noise_A)) / T)   [16384, 4096]
  G  = unsort(triu_strict_mask * sigmoid((logitexp(-0.5*d2(Y,Y)/exp(g)) + logit(noise_G)) / T))
with Y = uR sorted by sum(log Phi(uR)) and the output unsorted on both axes.

Device math (validated against the jax reference to ~2e-6 absmax):
  na  = k*|x|^2 + k*|y|^2 - 2k*(x.y)        (k = 0.5*exp(-g); via K=66 augmented matmul)
  lmc = ln(max(exp(na) - 1, 1e-20))         ( = -logitexp(-na) )
  w   = ln(u) - ln(1-u) - lmc
  out = sigmoid(w / T)

Sharding: A rows in contiguous 2048-row slabs per core; G computed in sorted
space with rows c::8 per core (balances the upper-triangular work), lower
block-triangle skipped (outputs are pre-zeroed by the runner), host applies the
inverse permutation on both axes afterwards.
"""

import math
from contextlib import ExitStack

import numpy as np

import concourse.bass as bass
import concourse.tile as tile
from concourse import mybir
from concourse.bass_utils import run_bass_kernel_spmd

M, N, D = 16384, 4096, 64
NCORES = 8
MLOC = M // NCORES          # 2048 A rows per core
GLOC = N // NCORES          # 512 G rows per core
KAUG = D + 2                # 66
P = 128                     # partitions
CT = 512                    # psum col tile (one bank of f32)
HW = 2048                   # elementwise half-block width
NBLK_A = MLOC // P          # 16
NBLK_G = GLOC // P          # 4
TEMP = 0.3
CHUNK = 4                   # blocks per ACT-table phase chunk

F32 = mybir.dt.float32
ACT = mybir.ActivationFunctionType
ALU = mybir.AluOpType

# column layout of the packed per-core matmul-constants tensor [KAUG, CW]
C_LHSA = 0                     # [:, :MLOC]   augmented uM slab, transposed
C_RHSA = C_LHSA + MLOC         # [:, :N]      augmented uR, transposed
C_LHSG = C_RHSA + N            # [:, :GLOC]   augmented sorted-Y slab (rows c::8)
C_RHSG = C_LHSG + GLOC         # [:, :N]      augmented sorted-Y, transposed
CW = C_RHSG + N

_PROGRAM = None


def _insts(x):
    return getattr(x, "ins", x)


def _patch_act_tables():
    # The act-table-load inserter greedily maps Exp -> exp_and_others and
    # Ln -> natural_log, paying a ~2.7us table switch on every Exp<->Ln
    # transition.  Strip exp/ln from every set except the combined
    # natural_log_exp_and_others (dict order preserved, so set ids still
    # match act_info.json) so both resolve to the one set.
    import concourse.bacc as bacc_mod

    orig = bacc_mod.get_activation_tables
    if getattr(bacc_mod.get_activation_tables, "_lnexp_patched", False):
        return
    both = {ACT.Exp, ACT.Ln}

    def patched(arch):
        t = orig(arch)
        return {
            name: (funcs if name == "natural_log_exp_and_others" else funcs - both)
            for name, funcs in t.items()
        }

    patched._lnexp_patched = True
    bacc_mod.get_activation_tables = patched


def _build_program():
    from concourse.bacc import Bacc

    _patch_act_tables()
    nc = Bacc()
    constsd = nc.declare_dram_parameter("consts", [KAUG, CW], F32, isOutput=False)
    maskd = nc.declare_dram_parameter("maskG", [P, 2 * CT], F32, isOutput=False)
    noiseA = nc.declare_dram_parameter("noiseA", [MLOC, N], F32, isOutput=False)
    noiseG = nc.declare_dram_parameter("noiseG", [GLOC, N], F32, isOutput=False)
    outA = nc.declare_dram_parameter("outA", [MLOC, N], F32, isOutput=True)
    outG = nc.declare_dram_parameter("outG", [GLOC, N], F32, isOutput=True)

    from concourse.tile_rust import add_dep_helper

    with tile.TileContext(nc) as tc, ExitStack() as ctx:
        consts = ctx.enter_context(tc.tile_pool(name="consts", bufs=1))
        consts_sb = consts.tile([KAUG, CW], F32)
        maskG_sb = consts.tile([P, 2 * CT], F32)
        g_consts_loaded = [False]

        def _load_g_consts():
            if not g_consts_loaded[0]:
                g_consts_loaded[0] = True
                nc.sync.dma_start(
                    out=consts_sb[:, C_LHSG:], in_=constsd[:, C_LHSG:]
                )
                nc.sync.dma_start(out=maskG_sb, in_=maskd[:, :])

        lhsA_sb = consts_sb[:, C_LHSA : C_LHSA + MLOC]
        rhsA_sb = consts_sb[:, C_RHSA : C_RHSA + N]
        lhsG_sb = consts_sb[:, C_LHSG : C_LHSG + GLOC]
        rhsG_sb = consts_sb[:, C_RHSG : C_RHSG + N]

        psum = ctx.enter_context(tc.tile_pool(name="psum", bufs=2, space="PSUM"))
        upool = ctx.enter_context(tc.tile_pool(name="u", bufs=3))
        mpool = ctx.enter_context(tc.tile_pool(name="m", bufs=3))
        wpool = ctx.enter_context(tc.tile_pool(name="w", bufs=CHUNK + 1))

        # A-matmul constants land first; the (colder) G constants and mask
        # are DMA'd lazily right before the first G block is traced.
        nc.sync.dma_start(out=consts_sb[:, :C_LHSG], in_=constsd[:, :C_LHSG])

        blocks = [("A", b) for b in range(NBLK_A)] + [("G", t) for t in range(NBLK_G)]
        prev_sig_insts = []
        for cstart in range(0, len(blocks), CHUNK):
            chunk = blocks[cstart : cstart + CHUNK]
            p1_act_insts = []
            todo = []
            for kind, b in chunk:
                if kind == "A":
                    col0 = 0
                    width = N
                    noise_dram = noiseA[b * P : (b + 1) * P, :]
                    lhs_sb = lhsA_sb[:, b * P : (b + 1) * P]
                    rhs_sb = rhsA_sb
                    out_dram = outA[b * P : (b + 1) * P, :]
                else:
                    _load_g_consts()
                    col0 = 2 * CT * b
                    width = N - col0
                    noise_dram = noiseG[b * P : (b + 1) * P, col0:]
                    lhs_sb = lhsG_sb[:, b * P : (b + 1) * P]
                    rhs_sb = rhsG_sb
                    out_dram = outG[b * P : (b + 1) * P, col0:]

                w = wpool.tile([P, width], F32, tag="w")
                h0 = 0
                while h0 < width:
                    hw = min(HW, width - h0)
                    wsl = w[:, h0 : h0 + hw]

                    u = upool.tile([P, hw], F32, tag="u")
                    nc.sync.dma_start(out=u, in_=noise_dram[:, h0 : h0 + hw])
                    # ln(u) goes straight into the w slice; the final combine
                    # subtracts ln((1-u)*mc) from it in place.
                    i1 = nc.scalar.activation(out=wsl, in_=u, func=ACT.Ln)

                    m = mpool.tile([P, hw], F32, tag="m")
                    exp_insts = []
                    ps = psum.tile([P, hw], F32, tag="ps")
                    for j in range(hw // CT):
                        gcol = col0 + h0 + j * CT
                        nc.tensor.matmul(
                            ps[:, j * CT : (j + 1) * CT],
                            lhs_sb,
                            rhs_sb[:, gcol : gcol + CT],
                            start=True,
                            stop=True,
                        )
                    ie = nc.scalar.activation(out=m, in_=ps, func=ACT.Exp)
                    exp_insts.append(ie)
                    # mc = max(exp(na)-1, eps); q = (u-1)*mc; lv = ln(-q) =
                    # ln((1-u)*mc) = ln(1-u) + ln(mc) -- one ACT ln instead of
                    # two, and the stt fuses the (u-1) with the multiply.
                    nc.vector.tensor_scalar(
                        out=m,
                        in0=m,
                        scalar1=1.0,
                        scalar2=1e-20,
                        op0=ALU.subtract,
                        op1=ALU.max,
                    )
                    nc.vector.scalar_tensor_tensor(
                        out=m,
                        in0=u,
                        scalar=1.0,
                        in1=m,
                        op0=ALU.subtract,
                        op1=ALU.mult,
                    )
                    i3 = nc.scalar.activation(out=m, in_=m, func=ACT.Ln, scale=-1.0)
                    nc.vector.tensor_tensor(out=wsl, in0=wsl, in1=m, op=ALU.subtract)

                    p1_act_insts += [i1, i3] + exp_insts
                    h0 += hw

                todo.append((kind, w, out_dram))

            # keep the ACT instruction stream grouped by table set:
            # every phase-1 (ln/exp) op of this chunk runs after every sigmoid
            # of the previous chunk, and every sigmoid of this chunk runs after
            # every phase-1 op of this chunk.
            for pi in p1_act_insts:
                for si in prev_sig_insts:
                    add_dep_helper(_insts(pi), _insts(si), True, "act-table phase")

            sig_insts = []
            for kind, w, out_dram in todo:
                if kind == "G":
                    # maskpen is +1e30 on kept entries, -1e30 on the masked
                    # lower triangle: min() then sigmoid gives exact-ish 0
                    # there, and keeps the ACT sigmoid as the final writer
                    # (out-DMA then needs a single sem wait).
                    nc.vector.tensor_tensor(
                        out=w[:, : 2 * CT],
                        in0=w[:, : 2 * CT],
                        in1=maskG_sb,
                        op=ALU.min,
                    )
                isig = nc.scalar.activation(
                    out=w, in_=w, func=ACT.Sigmoid, scale=float(1.0 / TEMP)
                )
                for pi in p1_act_insts:
                    add_dep_helper(_insts(isig), _insts(pi), True, "act-table phase")
                sig_insts.append(isig)
                nc.sync.dma_start(out=out_dram, in_=w)
            prev_sig_insts = sig_insts

    nc.finalize()
    return nc


def _get_program():
    global _PROGRAM
    if _PROGRAM is None:
        _PROGRAM = _build_program()
    return _PROGRAM


def _ordering_sort(uR):
    # Must match the reference bitwise: jax f32 on CPU, stable argsort.
    import jax
    import jax.numpy as jnp

    cpu = jax.devices("cpu")[0]
    with jax.default_device(cpu):
        ordering = jnp.sum(
            jnp.log(0.5 + 0.5 * jax.lax.erf(jnp.asarray(uR) / np.float32(math.sqrt(2.0)))),
            axis=1,
        )
        o = np.asarray(ordering)
    sort_idx = np.argsort(o, kind="stable")
    inv = np.argsort(sort_idx, kind="stable")
    return sort_idx, inv


def _augment(X, k):
    # lhs rows: [-2k*x, k*|x|^2, 1];  rhs rows: [y, 1, k*|y|^2]
    r = np.sum(X.astype(np.float32) ** 2, axis=1, dtype=np.float32)
    ones = np.ones((X.shape[0], 1), np.float32)
    lhs = np.concatenate([(np.float32(-2.0) * k) * X, (k * r)[:, None], ones], axis=1)
    rhs = np.concatenate([X, ones, (k * r)[:, None]], axis=1)
    return (
        np.ascontiguousarray(lhs.T.astype(np.float32)),
        np.ascontiguousarray(rhs.T.astype(np.float32)),
    )


def _pack_consts(lhsA_t, rhsA, lhsG_t, rhsG, c):
    out = np.empty((KAUG, CW), np.float32)
    out[:, C_LHSA : C_LHSA + MLOC] = lhsA_t[:, c * MLOC : (c + 1) * MLOC]
    out[:, C_RHSA : C_RHSA + N] = rhsA
    out[:, C_LHSG : C_LHSG + GLOC] = lhsG_t[:, c::NCORES]
    out[:, C_RHSG : C_RHSG + N] = rhsG
    return out


def _make_mask(c):
    # +1e30 on kept entries (strict upper triangle in sorted space for this
    # core's strided rows), -1e30 where masked; min() + sigmoid zeroes those.
    jj = np.arange(2 * CT, dtype=np.float32)[None, :]
    pp = np.arange(P, dtype=np.float32)[:, None]
    return np.where(jj > (c + 8.0 * pp), np.float32(1e30), np.float32(-1e30))


def make_in_maps(uM, uR, g_logscale, noise_A, noise_G):
    uM = np.asarray(uM, np.float32)
    uR = np.asarray(uR, np.float32)
    noise_A = np.asarray(noise_A, np.float32)
    noise_G = np.asarray(noise_G, np.float32)
    g = np.float32(np.asarray(g_logscale))

    sort_idx, inv = _ordering_sort(uR)
    k = np.float32(0.5) * np.exp(-g, dtype=np.float32)

    lhsA_t, _ = _augment(uM, k)             # [66, 16384]
    _, rhsA = _augment(uR, k)               # [66, 4096]
    Y = np.ascontiguousarray(uR[sort_idx])
    lhsG_t, rhsG = _augment(Y, k)           # [66, 4096], [66, 4096]

    in_maps = []
    for c in range(NCORES):
        in_maps.append(
            {
                "consts": _pack_consts(lhsA_t, rhsA, lhsG_t, rhsG, c),
                "maskG": np.ascontiguousarray(_make_mask(c)),
                "noiseA": np.ascontiguousarray(noise_A[c * MLOC : (c + 1) * MLOC]),
                "noiseG": np.ascontiguousarray(noise_G[c::NCORES]),
            }
        )
    return in_maps, inv


def assemble_outputs(results, inv):
    A = np.concatenate([results[c]["outA"] for c in range(NCORES)], axis=0)
    Gs = np.empty((N, N), np.float32)
    for c in range(NCORES):
        Gs[c::NCORES] = results[c]["outG"]
    G = np.ascontiguousarray(Gs[inv][:, inv])
    return A, G


def kernel(uM, uR, g_logscale, noise_A, noise_G, _trace=False):
    in_maps, inv = make_in_maps(uM, uR, g_logscale, noise_A, noise_G)
    nc = _get_program()
    res = run_bass_kernel_spmd(nc, in_maps, list(range(NCORES)), trace=_trace)
    A, G = assemble_outputs(res.results, inv)
    if _trace:
        return (A, G), res
    return A, G
